# revision 23
# baseline (speedup 1.0000x reference)
"""nn_AlignerOT distributed Trainium2 kernel (8 NeuronCores).

Per-token 1D entropic OT: 50 log-domain Sinkhorn iterations over per-token
[512,512] cost matrices cost = 300*(x_i - y_j)^2, then ot = mean_n(P)*D*SCALE
+ delta_ot and out = src @ ot.

Distribution: token axis (N=256) sharded 32/core across 8 cores; one AllReduce
of the [512,512] P-sum at the end; every core then computes its own output
shard with the replicated ot matrix.

Core tricks:
- The cost matrix is never materialized. The logsumexp argument
  g_j - 300(x_i-y_j)^2 - sigma_i is rank-3 in (i,j), so each tile of it is
  ONE K=12 TensorE matmul of bf16 3-limb decompositions (fp32-class accuracy
  at full PE speed).
- The logsumexp max-shift sigma is the previous iteration's logsumexp (a
  tight upper bound; per-iteration |dg| <= 0.23 after iteration 1, validated
  offline). A real max-reduce is only needed for 3 of the 100 passes.
- With sigma inside the matmul the exp needs no per-partition bias, so one
  ScalarE instruction exponentiates a whole token, amortizing the ~350-cycle
  ACT instruction overhead.
- Banding: x and y are sorted per token (host side). This is a 1D OT problem,
  so the transport plan concentrates near the sorted diagonal: every
  128-row i-tile only needs the static 256-wide j-window around its diagonal
  block (validated offline: windows of +-64 reproduce the full result to
  3e-6; +-32 does not). All matmuls/exps/sums run on [128,256] windows,
  halving all three engines.
- The final P accumulation runs full-width in ORIGINAL (unsorted)
  coordinates: the sorted-space sigma limbs are unsorted on-chip by tiny
  TensorE matmuls against host-provided 0/1 permutation matrices, after
  which mean_n P over tokens (each with its own sort) is well defined.
- Row sums of exp come from DVE tensor_reduce over the bf16 exp dump, with
  one token per half left on the ACT accum_out path to balance ACT vs DVE.
"""

import sys

sys.path.insert(0, "/opt/trn_rl_repo")

import numpy as np
import ml_dtypes

from concourse import bacc, tile, mybir
from concourse import hw_specs
from concourse.bass_utils import run_bass_kernel_spmd

F32 = mybir.dt.float32
BF16 = mybir.dt.bfloat16

REG = 0.1
SCALE = 300.0
D = 512
NCORES = 8
NTOK = 32            # tokens per core
NTOT = NCORES * NTOK
ITERS = 50
NACC = 1             # tokens per half-pass summed via ACT accum_out
W = 256              # banded window width per 128-row tile
LO = [0, 64, 192, 256]   # window start per tile (static; +-64 around diagonal)
SL = 256             # psum slot stride per window (bank-aligned)
RLA = float(REG * np.log(1.0 / D))
LA = float(np.log(1.0 / D))

# Force every activation onto the one table set that holds both Exp and Ln,
# so the compiler hoists a single ACT_TABLE_LOAD instead of thrashing
# exp_and_others <-> natural_log every pass. Indices into act_info.json must
# be preserved, so empty the other sets rather than removing them.
_orig_get_tables = hw_specs.get_activation_tables


def _patched_tables(arch):
    t = _orig_get_tables(arch)
    keep = "natural_log_exp_and_others"
    if keep in t:
        t = {k: (v if k == keep else set()) for k, v in t.items()}
    return t


hw_specs.get_activation_tables = _patched_tables
bacc.get_activation_tables = _patched_tables


def _limbs3(a):
    """f32 -> three bf16 limbs summing to ~f32 precision."""
    a = np.asarray(a, np.float32)
    l0 = a.astype(ml_dtypes.bfloat16)
    r1 = a - l0.astype(np.float32)
    l1 = r1.astype(ml_dtypes.bfloat16)
    r2 = r1 - l1.astype(np.float32)
    l2 = r2.astype(ml_dtypes.bfloat16)
    return l0, l1, l2


def _lhsT_host(v):
    """[NTOK,512] f32 -> [12,16384] bf16 rows [1,1,1,v0,v0,v0,v1,v1,v2,0,0,0].

    Rows 9-11 are the per-iteration sigma limbs (start at zero)."""
    v0, v1, v2 = _limbs3(v.reshape(-1))
    ones = np.ones(NTOK * 512, ml_dtypes.bfloat16)
    zero = np.zeros(NTOK * 512, ml_dtypes.bfloat16)
    return np.stack([ones, ones, ones, v0, v0, v0, v1, v1, v2, zero, zero, zero])


def _rhs_host(alpha, beta):
    """[12,16384] bf16 rows [a0,a1,a2,b0,b1,b2,b0,b1,b0,-1,-1,-1].

    Rows 9-11 multiply the lhsT sigma limbs: psum gets -sigma_i."""
    a0, a1, a2 = _limbs3(alpha.reshape(-1))
    b0, b1, b2 = _limbs3(beta.reshape(-1))
    mone = np.full(NTOK * 512, -1.0, ml_dtypes.bfloat16)
    return np.stack([a0, a1, a2, b0, b1, b2, b0, b1, b0, mone, mone, mone])


def _build(iters=ITERS):
    nc = bacc.Bacc("TRN2", target_bir_lowering=False, debug=False, num_devices=NCORES)

    lhsT1_e = nc.dram_tensor("lhsT1", [12, NTOK * 512], BF16, kind="ExternalInput")
    lhsT2_e = nc.dram_tensor("lhsT2", [12, NTOK * 512], BF16, kind="ExternalInput")
    rhs1_e = nc.dram_tensor("rhs1i", [12, NTOK * 512], BF16, kind="ExternalInput")
    rhs2_e = nc.dram_tensor("rhs2i", [12, NTOK * 512], BF16, kind="ExternalInput")
    lhsT1o_e = nc.dram_tensor("lhsT1o", [12, NTOK * 512], BF16, kind="ExternalInput")
    rhs1o_e = nc.dram_tensor("rhs1o", [12, NTOK * 512], BF16, kind="ExternalInput")
    permx_e = nc.dram_tensor("permx", [NTOK * 4 * 128, D], BF16, kind="ExternalInput")
    permy_e = nc.dram_tensor("permy", [NTOK * 4 * 128, D], BF16, kind="ExternalInput")
    xT_e = nc.dram_tensor("xT", [D, NTOK], F32, kind="ExternalInput")
    delta_e = nc.dram_tensor("delta", [D, D], F32, kind="ExternalInput")
    out_e = nc.dram_tensor("out", [NTOK, D], F32, kind="ExternalOutput")

    with tile.TileContext(nc, num_cores=NCORES) as tc:
        with (
            tc.tile_pool(name="state", bufs=1) as st,
            tc.tile_pool(name="work", bufs=2) as wk,
            tc.tile_pool(name="dumps", bufs=4) as dp,
            tc.tile_pool(name="psum", bufs=3, space="PSUM") as ps,
            tc.tile_pool(name="psum2", bufs=1, space="PSUM") as ps2,
            tc.tile_pool(name="dram", bufs=1, space="DRAM") as dr,
        ):
            lhsT = [st.tile([12, NTOK * 512], BF16, name=f"lhsT{p}") for p in range(2)]
            rhs = [st.tile([12, NTOK * 512], BF16, name=f"rhs{p}") for p in range(2)]
            sig = [st.tile([128, 128], F32, name=f"sig{p}") for p in range(2)]
            sigu = st.tile([128, 128], F32)
            biasc = st.tile([128, 128], F32)
            Scol = [st.tile([128, 128], F32, name=f"Scol{p}") for p in range(2)]
            Lcat = [st.tile([128, 384], BF16, name=f"Lcat{p}") for p in range(2)]
            Pacc = st.tile([128, 4 * D], F32)
            delta_sb = st.tile([128, 4 * D], F32)
            srcT = st.tile([128, 4 * NTOK], F32)
            ar_sb = st.tile([128, 4 * D], F32)
            out_sb = st.tile([NTOK, D], F32)

            nc.sync.dma_start(out=lhsT[0][:], in_=lhsT1_e.ap())
            nc.sync.dma_start(out=lhsT[1][:], in_=lhsT2_e.ap())
            nc.sync.dma_start(out=rhs[0][:], in_=rhs1_e.ap())
            nc.sync.dma_start(out=rhs[1][:], in_=rhs2_e.ap())
            for t in range(4):
                nc.sync.dma_start(out=srcT[:, t * NTOK : (t + 1) * NTOK],
                                  in_=xT_e.ap()[t * 128 : (t + 1) * 128, :])
                nc.sync.dma_start(out=delta_sb[:, t * D : (t + 1) * D],
                                  in_=delta_e.ap()[t * 128 : (t + 1) * 128, :])
            la_bias = st.tile([128, 1], F32)
            nc.vector.memset(la_bias[:], LA)
            nc.vector.memset(Pacc[:], 0.0)
            nc.vector.memset(sig[0][:], 0.0)
            nc.vector.memset(sig[1][:], 0.0)

            def emit_smalls(p, fresh, half, capture=False):
                """Per half (16 tokens = 64 columns): sigma' = sigma_in +
                [fresh max] + reg*ln(S); update the sigma limbs of lhsT[p] and
                the alpha limbs of rhs[1-p] for this half's flat range.
                capture=True additionally stores the limb columns interleaved
                into Lcat for the final unsort matmuls."""
                q = 1 - p
                c0, c1 = half * 64, (half + 1) * 64
                f0 = half * 8192
                sg = sig[p][:, c0:c1]
                lnS = wk.tile([128, 64], F32, tag="lnS", name="lnS")
                nc.scalar.activation(lnS[:], Scol[p][:, c0:c1], mybir.ActivationFunctionType.Ln)
                if fresh:
                    tmp = wk.tile([128, 64], F32, tag="tmp", name="tmp")
                    nc.vector.scalar_tensor_tensor(
                        out=tmp[:], in0=lnS[:], scalar=REG, in1=sigu[:, c0:c1],
                        op0=mybir.AluOpType.mult, op1=mybir.AluOpType.add)
                    nc.vector.tensor_tensor(sg, tmp[:], sg, mybir.AluOpType.add)
                else:
                    nc.vector.scalar_tensor_tensor(
                        out=sg, in0=lnS[:], scalar=REG, in1=sg,
                        op0=mybir.AluOpType.mult, op1=mybir.AluOpType.add)
                # alpha_other = RLA - sigma  (col-major)
                acm = wk.tile([128, 64], F32, tag="acm", name="acm")
                nc.vector.tensor_scalar(
                    out=acm[:], in0=sg, scalar1=-1.0, scalar2=RLA,
                    op0=mybir.AluOpType.mult, op1=mybir.AluOpType.add)
                # 3-limb split of alpha -> rhs[q] rows 0-2, and of sigma ->
                # lhsT[p] rows 9-11, via DMA xbar transpose + flatten.
                # capture: sigma1 limbs (p=0) and alpha1 limbs (p=1) feed the
                # final unsorted P pass.
                for src_cm, dst, base, cap in ((acm[:], rhs[q], 0, capture and p == 1),
                                               (sg, lhsT[p], 9, capture and p == 0)):
                    L0 = wk.tile([128, 128], BF16, tag="L0", name="L0")
                    L1 = wk.tile([128, 128], BF16, tag="L1", name="L1")
                    L2 = wk.tile([128, 128], BF16, tag="L2", name="L2")
                    R1 = wk.tile([128, 64], F32, tag="R1", name="R1")
                    R2 = wk.tile([128, 64], F32, tag="R2", name="R2")
                    nc.vector.tensor_copy(L0[:, c0:c1], src_cm)
                    nc.vector.tensor_tensor(R1[:], src_cm, L0[:, c0:c1], mybir.AluOpType.subtract)
                    nc.vector.tensor_copy(L1[:, c0:c1], R1[:])
                    nc.vector.tensor_tensor(R2[:], R1[:], L1[:, c0:c1], mybir.AluOpType.subtract)
                    nc.vector.tensor_copy(L2[:, c0:c1], R2[:])
                    for k, L in enumerate((L0, L1, L2)):
                        LT = wk.tile([128, 128], BF16, tag=f"LT{k}", name=f"LT{k}")
                        nc.sync.dma_start(out=LT[:], in_=L[:], transpose=True)
                        nc.sync.dma_start(out=dst[base + k : base + k + 1, f0 : f0 + 8192],
                                          in_=LT[c0:c1, :])
                        if cap:
                            pp = 0 if base == 9 else 1
                            nc.vector.tensor_copy(
                                Lcat[pp][:, 3 * c0 + k : 3 * c1 : 3], L[:, c0:c1])

            def emit_pass_fresh(p):
                """Peeled pass: banded, per-window exp with DVE max + AP bias +
                ACT accum (sigma rows of lhsT may hold a stale shift; the max
                is over the shifted psum, so sigma' = sigma_in + max + reg lnS)."""
                for half in range(2):
                    for n in range(half * 16, (half + 1) * 16):
                        pt = ps.tile([128, 1024], F32, tag="mm", name="pt")
                        for t in range(4):
                            col = n * 4 + t
                            nc.tensor.matmul(
                                pt[:, t * SL : t * SL + W],
                                lhsT[p][:, col * 128 : (col + 1) * 128],
                                rhs[p][:, n * 512 + LO[t] : n * 512 + LO[t] + W],
                                start=True, stop=True)
                        nc.vector.tensor_reduce(
                            sigu[:, n * 4 : (n + 1) * 4],
                            pt[:].rearrange("p (t f) -> p t f", t=4)[:, :, 0:W],
                            axis=mybir.AxisListType.X, op=mybir.AluOpType.max)
                        nc.vector.tensor_scalar(
                            out=biasc[:, n * 4 : (n + 1) * 4],
                            in0=sigu[:, n * 4 : (n + 1) * 4],
                            scalar1=-1.0 / REG, scalar2=None,
                            op0=mybir.AluOpType.mult)
                        for t in range(4):
                            col = n * 4 + t
                            dump = dp.tile([128, W], BF16, tag="dumpf", name="dumpf")
                            nc.scalar.activation(
                                dump[:], pt[:, t * SL : t * SL + W],
                                mybir.ActivationFunctionType.Exp,
                                bias=biasc[:, col : col + 1], scale=1.0 / REG,
                                accum_out=Scol[p][:, col : col + 1])
                    emit_smalls(p, fresh=True, half=half)

            def emit_pass(p, capture=False):
                """Steady-state pass: sigma shift inside the matmul, one
                FD=1024 exp per token (4 banded windows); sums on DVE (one
                grouped [128,4,W] reduce per token) except NACC tokens per
                half on ACT accum."""
                for half in range(2):
                    for n in range(half * 16, (half + 1) * 16):
                        pt = ps.tile([128, 1024], F32, tag="mm", name="pt")
                        for t in range(4):
                            nc.tensor.matmul(
                                pt[:, t * SL : t * SL + W],
                                lhsT[p][:, (n * 4 + t) * 128 : (n * 4 + t + 1) * 128],
                                rhs[p][:, n * 512 + LO[t] : n * 512 + LO[t] + W],
                                start=True, stop=True)
                        if n % 16 < NACC:
                            for t in range(4):
                                col = n * 4 + t
                                dump = dp.tile([128, W], BF16, tag="dumpf", name="dumpf")
                                nc.scalar.activation(
                                    dump[:], pt[:, t * SL : t * SL + W],
                                    mybir.ActivationFunctionType.Exp,
                                    scale=1.0 / REG,
                                    accum_out=Scol[p][:, col : col + 1])
                        else:
                            dump = dp.tile([128, 1024], BF16, tag="dump", name="dump")
                            nc.scalar.activation(
                                dump[:], pt[:], mybir.ActivationFunctionType.Exp,
                                scale=1.0 / REG)
                            # grouped reduce skips the 16-col pad of each slot
                            nc.vector.tensor_reduce(
                                Scol[p][:, n * 4 : (n + 1) * 4],
                                dump[:].rearrange("p (t f) -> p t f", t=4)[:, :, 0:W],
                                axis=mybir.AxisListType.X, op=mybir.AluOpType.add)
                    emit_smalls(p, fresh=False, half=half, capture=capture)

            # iterations 0,1 peeled: fresh max for pass1 of both and pass2 of 0
            emit_pass_fresh(0)
            emit_pass_fresh(1)
            emit_pass_fresh(0)
            emit_pass(1)
            if iters > 3:
                # seven iterations peeled so the remaining count divides 8,
                # then 8 iterations per hardware-loop body (fewer back-edges)
                for _ in range(7):
                    emit_pass(0)
                    emit_pass(1)
                with tc.For_i(9, iters - 1, 8, hint_engines=(mybir.EngineType.PE, mybir.EngineType.DVE, mybir.EngineType.Activation)):
                    for _ in range(8):
                        emit_pass(0)
                        emit_pass(1)
            # last iteration peeled to capture the final sigma/alpha limbs
            emit_pass(0, capture=True)
            emit_pass(1, capture=True)

            # sorted lhsT[0]/rhs[0] are dead now; reload them with the
            # unsorted-coordinate statics for the final P pass
            nc.sync.dma_start(out=lhsT[0][:], in_=lhsT1o_e.ap())
            nc.sync.dma_start(out=rhs[0][:], in_=rhs1o_e.ap())

            # ---- unsort sigma1/alpha1 limbs into original coordinates ----
            # out[l, j_orig] = sum_{j_s} limb_l[j_s] * Perm[j_s, j_orig]
            for n in range(NTOK):
                pxt = wk.tile([128, 4 * D], BF16, tag="pxt", name="pxt", bufs=3)
                pyt = wk.tile([128, 4 * D], BF16, tag="pyt", name="pyt", bufs=3)
                for t in range(4):
                    r0 = (n * 4 + t) * 128
                    nc.sync.dma_start(out=pxt[:, t * D : (t + 1) * D],
                                      in_=permx_e.ap()[r0 : r0 + 128, :])
                    nc.sync.dma_start(out=pyt[:, t * D : (t + 1) * D],
                                      in_=permy_e.ap()[r0 : r0 + 128, :])
                pot = ps2.tile([3, 1024], F32, tag="po", name="pot")
                po1 = pot[:, 0:D]
                po2 = pot[:, D : 2 * D]
                for t in range(4):
                    col = n * 4 + t
                    nc.tensor.matmul(po1, Lcat[0][:, 3 * col : 3 * col + 3],
                                     pxt[:, t * D : (t + 1) * D],
                                     start=(t == 0), stop=(t == 3))
                    nc.tensor.matmul(po2, Lcat[1][:, 3 * col : 3 * col + 3],
                                     pyt[:, t * D : (t + 1) * D],
                                     start=(t == 0), stop=(t == 3))
                stg = wk.tile([3, D], BF16, tag="stg", name="stg")
                nc.scalar.copy(stg[:], po1)
                nc.sync.dma_start(out=lhsT[0][9:12, n * D : (n + 1) * D], in_=stg[:])
                nc.scalar.copy(rhs[0][0:3, n * D : (n + 1) * D], po2)

            # final P accumulation, full width, original coordinates:
            # (f_i + g_j - c_ij)/reg = psum/reg + log(1/D) exactly.
            for n in range(NTOK):
                for h in range(2):
                    pt = ps.tile([128, 1024], F32, tag="mm", name="ptf")
                    for t in (2 * h, 2 * h + 1):
                        col = n * 4 + t
                        nc.tensor.matmul(
                            pt[:, (t % 2) * 512 : (t % 2 + 1) * 512],
                            lhsT[0][:, col * 128 : (col + 1) * 128],
                            rhs[0][:, n * 512 : (n + 1) * 512],
                            start=True, stop=True)
                    et = dp.tile([128, 1024], BF16, tag="dump", name="et")
                    nc.scalar.activation(et[:], pt[:], mybir.ActivationFunctionType.Exp,
                                         bias=la_bias[:], scale=1.0 / REG)
                    nc.vector.tensor_tensor(Pacc[:, h * 1024 : (h + 1) * 1024],
                                            Pacc[:, h * 1024 : (h + 1) * 1024],
                                            et[:], mybir.AluOpType.add)

            # AllReduce the P-sum across the 8 cores
            ccin = dr.tile([D, D], F32)
            ccout = dr.tile([D, D], F32, addr_space="Shared")
            for t in range(4):
                nc.sync.dma_start(out=ccin[:][t * 128 : (t + 1) * 128, :],
                                  in_=Pacc[:, t * D : (t + 1) * D])
            nc.gpsimd.collective_compute(
                "AllReduce", mybir.AluOpType.add,
                replica_groups=[list(range(NCORES))],
                ins=[ccin[:].opt()], outs=[ccout[:].opt()])
            for t in range(4):
                nc.sync.dma_start(out=ar_sb[:, t * D : (t + 1) * D],
                                  in_=ccout[:][t * 128 : (t + 1) * 128, :])
            # ot = ar * (D*SCALE/NTOT) + delta
            nc.vector.scalar_tensor_tensor(
                out=ar_sb[:], in0=ar_sb[:], scalar=float(D * SCALE / NTOT),
                in1=delta_sb[:], op0=mybir.AluOpType.mult, op1=mybir.AluOpType.add)
            # out = src @ ot   (fp32 matmuls, K=128 per i-tile)
            po = ps.tile([128, 1024], F32, tag="mm", name="po")
            for t in range(4):
                nc.tensor.matmul(
                    po[0:NTOK, 0:D],
                    srcT[:, t * NTOK : (t + 1) * NTOK],
                    ar_sb[:, t * D : (t + 1) * D],
                    start=(t == 0), stop=(t == 3))
            nc.vector.tensor_copy(out_sb[:], po[0:NTOK, 0:D])
            nc.sync.dma_start(out=out_e.ap(), in_=out_sb[:])

    nc.compile()
    return nc


def _host_inputs(X, Y, delta_ot):
    """Build the 8 per-core input maps from the full problem inputs."""
    src = np.ascontiguousarray(X.reshape(-1, D).astype(np.float32))
    tgt = np.ascontiguousarray(Y.reshape(-1, D).astype(np.float32))
    delta = np.ascontiguousarray(delta_ot.astype(np.float32))
    maps = []
    for c in range(NCORES):
        x = src[c * NTOK : (c + 1) * NTOK]
        y = tgt[c * NTOK : (c + 1) * NTOK]
        xi = np.argsort(x, axis=1)
        yi = np.argsort(y, axis=1)
        xs = np.take_along_axis(x, xi, axis=1)
        ys = np.take_along_axis(y, yi, axis=1)
        # permutation matrices: Perm[sorted_pos, orig_pos] = 1
        permx = np.zeros((NTOK, D, D), ml_dtypes.bfloat16)
        permy = np.zeros((NTOK, D, D), ml_dtypes.bfloat16)
        rows = np.arange(D)
        for n in range(NTOK):
            permx[n, rows, xi[n]] = 1
            permy[n, rows, yi[n]] = 1
        maps.append({
            "lhsT1": np.ascontiguousarray(_lhsT_host(xs)).view(np.uint16),
            "lhsT2": np.ascontiguousarray(_lhsT_host(ys)).view(np.uint16),
            "rhs1i": np.ascontiguousarray(_rhs_host(-SCALE * ys * ys, 600.0 * ys)).view(np.uint16),
            "rhs2i": np.ascontiguousarray(_rhs_host(np.zeros_like(xs), 600.0 * xs)).view(np.uint16),
            "lhsT1o": np.ascontiguousarray(_lhsT_host(x)).view(np.uint16),
            "rhs1o": np.ascontiguousarray(_rhs_host(np.zeros_like(y), 600.0 * y)).view(np.uint16),
            "permx": np.ascontiguousarray(permx.reshape(NTOK * D, D)).view(np.uint16),
            "permy": np.ascontiguousarray(permy.reshape(NTOK * D, D)).view(np.uint16),
            "xT": np.ascontiguousarray(x.T),
            "delta": delta,
        })
    return maps


_cache = {}


def _get_nc(iters=ITERS):
    if iters not in _cache:
        _cache[iters] = _build(iters)
    return _cache[iters]


def kernel(X, Y, delta_ot, _iters=ITERS, _trace=False):
    nc = _get_nc(_iters)
    maps = _host_inputs(np.asarray(X), np.asarray(Y), np.asarray(delta_ot))
    res = run_bass_kernel_spmd(nc, maps, list(range(NCORES)), trace=_trace)
    out = np.concatenate([res.results[c]["out"] for c in range(NCORES)], axis=0)
    B, S = 2, 128
    out = out.reshape(B, S, D).astype(np.float32)
    if _trace:
        return out, res
    return out


# revision 24
# speedup vs baseline: 1.1704x; 1.1704x over previous
"""nn_AlignerOT distributed Trainium2 kernel (8 NeuronCores).

Per-token 1D entropic OT: 50 log-domain Sinkhorn iterations over per-token
[512,512] cost matrices cost = 300*(x_i - y_j)^2, then ot = mean_n(P)*D*SCALE
+ delta_ot and out = src @ ot.

Distribution: token axis (N=256) sharded 32/core across 8 cores; one AllReduce
of the [512,512] P-sum at the end; every core then computes its own output
shard with the replicated ot matrix.

Core tricks:
- The cost matrix is never materialized. The logsumexp argument
  g_j - 300(x_i-y_j)^2 - sigma_i is rank-3 in (i,j), so each tile of it is
  ONE K=12 TensorE matmul of bf16 3-limb decompositions (fp32-class accuracy
  at full PE speed).
- The logsumexp max-shift sigma is the previous iteration's logsumexp (a
  tight upper bound; per-iteration |dg| <= 0.23 after iteration 1, validated
  offline). A real max-reduce is only needed for 3 of the 100 passes.
- With sigma inside the matmul the exp needs no per-partition bias, so one
  ScalarE instruction exponentiates a whole token, amortizing the ~350-cycle
  ACT instruction overhead.
- Banding: x and y are sorted per token (host side). This is a 1D OT problem,
  so the transport plan concentrates near the sorted diagonal: every
  128-row i-tile only needs the static 256-wide j-window around its diagonal
  block (validated offline: windows of +-64 reproduce the full result to
  3e-6; +-32 does not). All matmuls/exps/sums run on [128,256] windows,
  halving all three engines.
- The final P accumulation runs full-width in ORIGINAL (unsorted)
  coordinates: the sorted-space sigma limbs are unsorted on-chip by tiny
  TensorE matmuls against host-provided 0/1 permutation matrices, after
  which mean_n P over tokens (each with its own sort) is well defined.
- Row sums of exp come from DVE tensor_reduce over the bf16 exp dump, with
  one token per half left on the ACT accum_out path to balance ACT vs DVE.
"""

import sys

sys.path.insert(0, "/opt/trn_rl_repo")

import numpy as np
import ml_dtypes

from concourse import bacc, tile, mybir
from concourse import hw_specs
from concourse.bass_utils import run_bass_kernel_spmd

F32 = mybir.dt.float32
BF16 = mybir.dt.bfloat16

REG = 0.1
SCALE = 300.0
D = 512
NCORES = 8
NTOK = 32            # tokens per core
NTOT = NCORES * NTOK
ITERS = 50
NACC = 1             # tokens per half-pass summed via ACT accum_out
W = 256              # banded window width per 128-row tile
LO = [0, 64, 192, 256]   # window start per tile (static; +-64 around diagonal)
SL = 256             # psum slot stride per window (bank-aligned)
RLA = float(REG * np.log(1.0 / D))
LA = float(np.log(1.0 / D))

# Force every activation onto the one table set that holds both Exp and Ln,
# so the compiler hoists a single ACT_TABLE_LOAD instead of thrashing
# exp_and_others <-> natural_log every pass. Indices into act_info.json must
# be preserved, so empty the other sets rather than removing them.
_orig_get_tables = hw_specs.get_activation_tables


def _patched_tables(arch):
    t = _orig_get_tables(arch)
    keep = "natural_log_exp_and_others"
    if keep in t:
        t = {k: (v if k == keep else set()) for k, v in t.items()}
    return t


hw_specs.get_activation_tables = _patched_tables
bacc.get_activation_tables = _patched_tables


def _limbs3(a):
    """f32 -> three bf16 limbs summing to ~f32 precision."""
    a = np.asarray(a, np.float32)
    l0 = a.astype(ml_dtypes.bfloat16)
    r1 = a - l0.astype(np.float32)
    l1 = r1.astype(ml_dtypes.bfloat16)
    r2 = r1 - l1.astype(np.float32)
    l2 = r2.astype(ml_dtypes.bfloat16)
    return l0, l1, l2


def _lhsT_host(v):
    """[NTOK,512] f32 -> [12,16384] bf16 rows [1,1,1,v0,v0,v0,v1,v1,v2,0,0,0].

    Rows 9-11 are the per-iteration sigma limbs (start at zero)."""
    v0, v1, v2 = _limbs3(v.reshape(-1))
    ones = np.ones(NTOK * 512, ml_dtypes.bfloat16)
    zero = np.zeros(NTOK * 512, ml_dtypes.bfloat16)
    return np.stack([ones, ones, ones, v0, v0, v0, v1, v1, v2, zero, zero, zero])


def _rhs_host(alpha, beta):
    """[12,16384] bf16 rows [a0,a1,a2,b0,b1,b2,b0,b1,b0,-1,-1,-1].

    Rows 9-11 multiply the lhsT sigma limbs: psum gets -sigma_i."""
    a0, a1, a2 = _limbs3(alpha.reshape(-1))
    b0, b1, b2 = _limbs3(beta.reshape(-1))
    mone = np.full(NTOK * 512, -1.0, ml_dtypes.bfloat16)
    return np.stack([a0, a1, a2, b0, b1, b2, b0, b1, b0, mone, mone, mone])


def _build(iters=ITERS):
    nc = bacc.Bacc("TRN2", target_bir_lowering=False, debug=False, num_devices=NCORES)

    lhsT1_e = nc.dram_tensor("lhsT1", [12, NTOK * 512], BF16, kind="ExternalInput")
    lhsT2_e = nc.dram_tensor("lhsT2", [12, NTOK * 512], BF16, kind="ExternalInput")
    rhs1_e = nc.dram_tensor("rhs1i", [12, NTOK * 512], BF16, kind="ExternalInput")
    rhs2_e = nc.dram_tensor("rhs2i", [12, NTOK * 512], BF16, kind="ExternalInput")
    lhsT1o_e = nc.dram_tensor("lhsT1o", [12, NTOK * 512], BF16, kind="ExternalInput")
    rhs1o_e = nc.dram_tensor("rhs1o", [12, NTOK * 512], BF16, kind="ExternalInput")
    permx_e = nc.dram_tensor("permx", [NTOK * 4 * 128, D], BF16, kind="ExternalInput")
    permy_e = nc.dram_tensor("permy", [NTOK * 4 * 128, D], BF16, kind="ExternalInput")
    xT_e = nc.dram_tensor("xT", [D, NTOK], F32, kind="ExternalInput")
    delta_e = nc.dram_tensor("delta", [D, D], F32, kind="ExternalInput")
    out_e = nc.dram_tensor("out", [NTOK, D], F32, kind="ExternalOutput")

    with tile.TileContext(nc, num_cores=NCORES) as tc:
        with (
            tc.tile_pool(name="state", bufs=1) as st,
            tc.tile_pool(name="work", bufs=2) as wk,
            tc.tile_pool(name="dumps", bufs=4) as dp,
            tc.tile_pool(name="psum", bufs=3, space="PSUM") as ps,
            tc.tile_pool(name="psum2", bufs=1, space="PSUM") as ps2,
            tc.tile_pool(name="dram", bufs=1, space="DRAM") as dr,
        ):
            lhsT = [st.tile([12, NTOK * 512], BF16, name=f"lhsT{p}") for p in range(2)]
            rhs = [st.tile([12, NTOK * 512], BF16, name=f"rhs{p}") for p in range(2)]
            sig = [st.tile([128, 128], F32, name=f"sig{p}") for p in range(2)]
            sigu = st.tile([128, 128], F32)
            biasc = st.tile([128, 128], F32)
            Scol = [st.tile([128, 128], F32, name=f"Scol{p}") for p in range(2)]
            Lcat = [st.tile([128, 384], BF16, name=f"Lcat{p}") for p in range(2)]
            Pacc = st.tile([128, 4 * D], F32)
            delta_sb = st.tile([128, 4 * D], F32)
            srcT = st.tile([128, 4 * NTOK], F32)
            ar_sb = st.tile([128, 4 * D], F32)
            out_sb = st.tile([NTOK, D], F32)

            nc.sync.dma_start(out=lhsT[0][:], in_=lhsT1_e.ap())
            nc.sync.dma_start(out=lhsT[1][:], in_=lhsT2_e.ap())
            nc.sync.dma_start(out=rhs[0][:], in_=rhs1_e.ap())
            nc.sync.dma_start(out=rhs[1][:], in_=rhs2_e.ap())
            for t in range(4):
                nc.sync.dma_start(out=srcT[:, t * NTOK : (t + 1) * NTOK],
                                  in_=xT_e.ap()[t * 128 : (t + 1) * 128, :])
                nc.sync.dma_start(out=delta_sb[:, t * D : (t + 1) * D],
                                  in_=delta_e.ap()[t * 128 : (t + 1) * 128, :])
            la_bias = st.tile([128, 1], F32)
            nc.vector.memset(la_bias[:], LA)
            nc.vector.memset(Pacc[:], 0.0)
            nc.vector.memset(sig[0][:], 0.0)
            nc.vector.memset(sig[1][:], 0.0)

            def emit_smalls(p, fresh, half, capture=False):
                """Per half (16 tokens = 64 columns): sigma' = sigma_in +
                [fresh max] + reg*ln(S); update the sigma limbs of lhsT[p] and
                the alpha limbs of rhs[1-p] for this half's flat range.
                capture=True additionally stores the limb columns interleaved
                into Lcat for the final unsort matmuls."""
                q = 1 - p
                c0, c1 = half * 64, (half + 1) * 64
                f0 = half * 8192
                sg = sig[p][:, c0:c1]
                lnS = wk.tile([128, 64], F32, tag="lnS", name="lnS")
                nc.scalar.activation(lnS[:], Scol[p][:, c0:c1], mybir.ActivationFunctionType.Ln)
                if fresh:
                    tmp = wk.tile([128, 64], F32, tag="tmp", name="tmp")
                    nc.vector.scalar_tensor_tensor(
                        out=tmp[:], in0=lnS[:], scalar=REG, in1=sigu[:, c0:c1],
                        op0=mybir.AluOpType.mult, op1=mybir.AluOpType.add)
                    nc.vector.tensor_tensor(sg, tmp[:], sg, mybir.AluOpType.add)
                else:
                    nc.vector.scalar_tensor_tensor(
                        out=sg, in0=lnS[:], scalar=REG, in1=sg,
                        op0=mybir.AluOpType.mult, op1=mybir.AluOpType.add)
                # alpha_other = RLA - sigma  (col-major)
                acm = wk.tile([128, 64], F32, tag="acm", name="acm")
                nc.vector.tensor_scalar(
                    out=acm[:], in0=sg, scalar1=-1.0, scalar2=RLA,
                    op0=mybir.AluOpType.mult, op1=mybir.AluOpType.add)
                # 3-limb split of alpha -> rhs[q] rows 0-2, and of sigma ->
                # lhsT[p] rows 9-11, via DMA xbar transpose + flatten.
                # capture: sigma1 limbs (p=0) and alpha1 limbs (p=1) feed the
                # final unsorted P pass.
                for src_cm, dst, base, cap in ((acm[:], rhs[q], 0, capture and p == 1),
                                               (sg, lhsT[p], 9, capture and p == 0)):
                    L0 = wk.tile([128, 128], BF16, tag="L0", name="L0")
                    L1 = wk.tile([128, 128], BF16, tag="L1", name="L1")
                    L2 = wk.tile([128, 128], BF16, tag="L2", name="L2")
                    R1 = wk.tile([128, 64], F32, tag="R1", name="R1")
                    R2 = wk.tile([128, 64], F32, tag="R2", name="R2")
                    nc.vector.tensor_copy(L0[:, c0:c1], src_cm)
                    nc.vector.tensor_tensor(R1[:], src_cm, L0[:, c0:c1], mybir.AluOpType.subtract)
                    nc.vector.tensor_copy(L1[:, c0:c1], R1[:])
                    nc.vector.tensor_tensor(R2[:], R1[:], L1[:, c0:c1], mybir.AluOpType.subtract)
                    nc.vector.tensor_copy(L2[:, c0:c1], R2[:])
                    for k, L in enumerate((L0, L1, L2)):
                        LT = wk.tile([128, 128], BF16, tag=f"LT{k}", name=f"LT{k}")
                        nc.sync.dma_start(out=LT[:], in_=L[:], transpose=True)
                        nc.sync.dma_start(out=dst[base + k : base + k + 1, f0 : f0 + 8192],
                                          in_=LT[c0:c1, :])
                        if cap:
                            pp = 0 if base == 9 else 1
                            nc.vector.tensor_copy(
                                Lcat[pp][:, 3 * c0 + k : 3 * c1 : 3], L[:, c0:c1])

            def emit_pass_fresh(p):
                """Peeled pass: banded, per-window exp with DVE max + AP bias +
                ACT accum (sigma rows of lhsT may hold a stale shift; the max
                is over the shifted psum, so sigma' = sigma_in + max + reg lnS)."""
                for half in range(2):
                    for n in range(half * 16, (half + 1) * 16):
                        pt = ps.tile([128, 1024], F32, tag="mm", name="pt")
                        for t in range(4):
                            col = n * 4 + t
                            nc.tensor.matmul(
                                pt[:, t * SL : t * SL + W],
                                lhsT[p][:, col * 128 : (col + 1) * 128],
                                rhs[p][:, n * 512 + LO[t] : n * 512 + LO[t] + W],
                                start=True, stop=True)
                        nc.vector.tensor_reduce(
                            sigu[:, n * 4 : (n + 1) * 4],
                            pt[:].rearrange("p (t f) -> p t f", t=4)[:, :, 0:W],
                            axis=mybir.AxisListType.X, op=mybir.AluOpType.max)
                        nc.vector.tensor_scalar(
                            out=biasc[:, n * 4 : (n + 1) * 4],
                            in0=sigu[:, n * 4 : (n + 1) * 4],
                            scalar1=-1.0 / REG, scalar2=None,
                            op0=mybir.AluOpType.mult)
                        for t in range(4):
                            col = n * 4 + t
                            dump = dp.tile([128, W], BF16, tag="dumpf", name="dumpf")
                            nc.scalar.activation(
                                dump[:], pt[:, t * SL : t * SL + W],
                                mybir.ActivationFunctionType.Exp,
                                bias=biasc[:, col : col + 1], scale=1.0 / REG,
                                accum_out=Scol[p][:, col : col + 1])
                    emit_smalls(p, fresh=True, half=half)

            def emit_pass(p, capture=False):
                """Steady-state pass: sigma shift inside the matmul, one
                FD=1024 exp per token (4 banded windows); sums on DVE (one
                grouped [128,4,W] reduce per token) except NACC tokens per
                half on ACT accum."""
                for half in range(2):
                    for n in range(half * 16, (half + 1) * 16):
                        pt = ps.tile([128, 1024], F32, tag="mm", name="pt")
                        for t in range(4):
                            nc.tensor.matmul(
                                pt[:, t * SL : t * SL + W],
                                lhsT[p][:, (n * 4 + t) * 128 : (n * 4 + t + 1) * 128],
                                rhs[p][:, n * 512 + LO[t] : n * 512 + LO[t] + W],
                                start=True, stop=True)
                        if n % 16 < NACC:
                            for t in range(4):
                                col = n * 4 + t
                                dump = dp.tile([128, W], BF16, tag="dumpf", name="dumpf")
                                nc.scalar.activation(
                                    dump[:], pt[:, t * SL : t * SL + W],
                                    mybir.ActivationFunctionType.Exp,
                                    scale=1.0 / REG,
                                    accum_out=Scol[p][:, col : col + 1])
                        else:
                            dump = dp.tile([128, 1024], BF16, tag="dump", name="dump")
                            nc.scalar.activation(
                                dump[:], pt[:], mybir.ActivationFunctionType.Exp,
                                scale=1.0 / REG)
                            # grouped reduce skips the 16-col pad of each slot
                            nc.vector.tensor_reduce(
                                Scol[p][:, n * 4 : (n + 1) * 4],
                                dump[:].rearrange("p (t f) -> p t f", t=4)[:, :, 0:W],
                                axis=mybir.AxisListType.X, op=mybir.AluOpType.add)
                    emit_smalls(p, fresh=False, half=half, capture=capture)

            # iterations 0,1 peeled: fresh max for pass1 of both and pass2 of 0
            emit_pass_fresh(0)
            emit_pass_fresh(1)
            emit_pass_fresh(0)
            emit_pass(1)
            if iters > 3:
                # three iterations peeled so the remaining count divides 4,
                # then 4 iterations per hardware-loop body (fewer back-edges;
                # 8-iteration bodies overflow IRAM and regress)
                for _ in range(3):
                    emit_pass(0)
                    emit_pass(1)
                with tc.For_i(5, iters - 1, 4, hint_engines=(mybir.EngineType.PE, mybir.EngineType.DVE, mybir.EngineType.Activation)):
                    for _ in range(4):
                        emit_pass(0)
                        emit_pass(1)
            # last iteration peeled to capture the final sigma/alpha limbs
            emit_pass(0, capture=True)
            emit_pass(1, capture=True)

            # sorted lhsT[0]/rhs[0] are dead now; reload them with the
            # unsorted-coordinate statics for the final P pass
            nc.sync.dma_start(out=lhsT[0][:], in_=lhsT1o_e.ap())
            nc.sync.dma_start(out=rhs[0][:], in_=rhs1o_e.ap())

            # ---- unsort sigma1/alpha1 limbs into original coordinates ----
            # out[l, j_orig] = sum_{j_s} limb_l[j_s] * Perm[j_s, j_orig]
            for n in range(NTOK):
                pxt = wk.tile([128, 4 * D], BF16, tag="pxt", name="pxt", bufs=3)
                pyt = wk.tile([128, 4 * D], BF16, tag="pyt", name="pyt", bufs=3)
                for t in range(4):
                    r0 = (n * 4 + t) * 128
                    nc.sync.dma_start(out=pxt[:, t * D : (t + 1) * D],
                                      in_=permx_e.ap()[r0 : r0 + 128, :])
                    nc.sync.dma_start(out=pyt[:, t * D : (t + 1) * D],
                                      in_=permy_e.ap()[r0 : r0 + 128, :])
                pot = ps2.tile([3, 1024], F32, tag="po", name="pot")
                po1 = pot[:, 0:D]
                po2 = pot[:, D : 2 * D]
                for t in range(4):
                    col = n * 4 + t
                    nc.tensor.matmul(po1, Lcat[0][:, 3 * col : 3 * col + 3],
                                     pxt[:, t * D : (t + 1) * D],
                                     start=(t == 0), stop=(t == 3))
                    nc.tensor.matmul(po2, Lcat[1][:, 3 * col : 3 * col + 3],
                                     pyt[:, t * D : (t + 1) * D],
                                     start=(t == 0), stop=(t == 3))
                stg = wk.tile([3, D], BF16, tag="stg", name="stg")
                nc.scalar.copy(stg[:], po1)
                nc.sync.dma_start(out=lhsT[0][9:12, n * D : (n + 1) * D], in_=stg[:])
                nc.scalar.copy(rhs[0][0:3, n * D : (n + 1) * D], po2)

            # final P accumulation, full width, original coordinates:
            # (f_i + g_j - c_ij)/reg = psum/reg + log(1/D) exactly.
            for n in range(NTOK):
                for h in range(2):
                    pt = ps.tile([128, 1024], F32, tag="mm", name="ptf")
                    for t in (2 * h, 2 * h + 1):
                        col = n * 4 + t
                        nc.tensor.matmul(
                            pt[:, (t % 2) * 512 : (t % 2 + 1) * 512],
                            lhsT[0][:, col * 128 : (col + 1) * 128],
                            rhs[0][:, n * 512 : (n + 1) * 512],
                            start=True, stop=True)
                    et = dp.tile([128, 1024], BF16, tag="dump", name="et")
                    nc.scalar.activation(et[:], pt[:], mybir.ActivationFunctionType.Exp,
                                         bias=la_bias[:], scale=1.0 / REG)
                    nc.vector.tensor_tensor(Pacc[:, h * 1024 : (h + 1) * 1024],
                                            Pacc[:, h * 1024 : (h + 1) * 1024],
                                            et[:], mybir.AluOpType.add)

            # AllReduce the P-sum across the 8 cores
            ccin = dr.tile([D, D], F32)
            ccout = dr.tile([D, D], F32, addr_space="Shared")
            for t in range(4):
                nc.sync.dma_start(out=ccin[:][t * 128 : (t + 1) * 128, :],
                                  in_=Pacc[:, t * D : (t + 1) * D])
            nc.gpsimd.collective_compute(
                "AllReduce", mybir.AluOpType.add,
                replica_groups=[list(range(NCORES))],
                ins=[ccin[:].opt()], outs=[ccout[:].opt()])
            for t in range(4):
                nc.sync.dma_start(out=ar_sb[:, t * D : (t + 1) * D],
                                  in_=ccout[:][t * 128 : (t + 1) * 128, :])
            # ot = ar * (D*SCALE/NTOT) + delta
            nc.vector.scalar_tensor_tensor(
                out=ar_sb[:], in0=ar_sb[:], scalar=float(D * SCALE / NTOT),
                in1=delta_sb[:], op0=mybir.AluOpType.mult, op1=mybir.AluOpType.add)
            # out = src @ ot   (fp32 matmuls, K=128 per i-tile)
            po = ps.tile([128, 1024], F32, tag="mm", name="po")
            for t in range(4):
                nc.tensor.matmul(
                    po[0:NTOK, 0:D],
                    srcT[:, t * NTOK : (t + 1) * NTOK],
                    ar_sb[:, t * D : (t + 1) * D],
                    start=(t == 0), stop=(t == 3))
            nc.vector.tensor_copy(out_sb[:], po[0:NTOK, 0:D])
            nc.sync.dma_start(out=out_e.ap(), in_=out_sb[:])

    nc.compile()
    return nc


def _host_inputs(X, Y, delta_ot):
    """Build the 8 per-core input maps from the full problem inputs."""
    src = np.ascontiguousarray(X.reshape(-1, D).astype(np.float32))
    tgt = np.ascontiguousarray(Y.reshape(-1, D).astype(np.float32))
    delta = np.ascontiguousarray(delta_ot.astype(np.float32))
    maps = []
    for c in range(NCORES):
        x = src[c * NTOK : (c + 1) * NTOK]
        y = tgt[c * NTOK : (c + 1) * NTOK]
        xi = np.argsort(x, axis=1)
        yi = np.argsort(y, axis=1)
        xs = np.take_along_axis(x, xi, axis=1)
        ys = np.take_along_axis(y, yi, axis=1)
        # permutation matrices: Perm[sorted_pos, orig_pos] = 1
        permx = np.zeros((NTOK, D, D), ml_dtypes.bfloat16)
        permy = np.zeros((NTOK, D, D), ml_dtypes.bfloat16)
        rows = np.arange(D)
        for n in range(NTOK):
            permx[n, rows, xi[n]] = 1
            permy[n, rows, yi[n]] = 1
        maps.append({
            "lhsT1": np.ascontiguousarray(_lhsT_host(xs)).view(np.uint16),
            "lhsT2": np.ascontiguousarray(_lhsT_host(ys)).view(np.uint16),
            "rhs1i": np.ascontiguousarray(_rhs_host(-SCALE * ys * ys, 600.0 * ys)).view(np.uint16),
            "rhs2i": np.ascontiguousarray(_rhs_host(np.zeros_like(xs), 600.0 * xs)).view(np.uint16),
            "lhsT1o": np.ascontiguousarray(_lhsT_host(x)).view(np.uint16),
            "rhs1o": np.ascontiguousarray(_rhs_host(np.zeros_like(y), 600.0 * y)).view(np.uint16),
            "permx": np.ascontiguousarray(permx.reshape(NTOK * D, D)).view(np.uint16),
            "permy": np.ascontiguousarray(permy.reshape(NTOK * D, D)).view(np.uint16),
            "xT": np.ascontiguousarray(x.T),
            "delta": delta,
        })
    return maps


_cache = {}


def _get_nc(iters=ITERS):
    if iters not in _cache:
        _cache[iters] = _build(iters)
    return _cache[iters]


def kernel(X, Y, delta_ot, _iters=ITERS, _trace=False):
    nc = _get_nc(_iters)
    maps = _host_inputs(np.asarray(X), np.asarray(Y), np.asarray(delta_ot))
    res = run_bass_kernel_spmd(nc, maps, list(range(NCORES)), trace=_trace)
    out = np.concatenate([res.results[c]["out"] for c in range(NCORES)], axis=0)
    B, S = 2, 128
    out = out.reshape(B, S, D).astype(np.float32)
    if _trace:
        return out, res
    return out
